# revision 1
# baseline (speedup 1.0000x reference)
"""HEALPix downsample (scatter-mean over parent_map + 1x1 conv) on 8 Trainium2 cores.

Strategy (matches the data-parallel sharding hint):
  - Data-parallel over the 16 flattened (B*C*T) slices: 2 slices per core.
  - Host prep: sort source pixels by parent, group by 256-target windows,
    pad each group to whole 128-row sub-tiles, pre-pack x so every DMA is
    a big linear per-partition read.
  - Device: scatter-mean is done on the TensorEngine: for each 128-row
    sub-tile, build a one-hot routing matrix A[row, tgt] = inv_count(parent)
    on the VectorEngine (iota == rel) * wgt, then psum[d, m] += X^T @ A
    accumulated over the group's sub-tiles. The 1x1 conv is a second matmul
    with W^T stationary: out[e, m] = (W^T)^T @ mean. Bias is fused into the
    PSUM->SBUF copy on the ScalarEngine (per-partition bias add).
  - Output is produced as [e, m] per slice and transposed back on the host.
"""

import numpy as np
from contextlib import ExitStack

import concourse.bacc as bacc
import concourse.tile as tile
from concourse import mybir
from concourse.bass_utils import run_bass_kernel_spmd

# ---- problem constants (hardcoded; kernel.py must be self-contained) ----
B, C, T = 2, 2, 4
S = B * C * T                 # 16 flattened batch slices
N_SRC = 49152
N_TGT = 12288
D = 128
N_CORES = 8
S_PER_CORE = S // N_CORES     # 2

P = 128                       # partitions
G = 256                       # targets per psum window (one group)
BW = 64                       # narrow band width for the scatter matmul
CHUNK_SUBS = 64               # sub-tiles per x DMA chunk (64*128 cols * 512B = 4 MB)
OUT_GROUPS = 8                # groups staged per output DMA (8*256 tgt * 512B = 1 MB)

F32 = mybir.dt.float32


def _plan(parent_map):
    """Host-side metadata: sorted/grouped row order, per-row (rel, wgt), bands."""
    pm = np.asarray(parent_map).astype(np.int64).ravel()
    assert pm.shape == (N_SRC,)
    perm = np.argsort(pm, kind="stable")
    sp = pm[perm]
    cnt = np.bincount(pm, minlength=N_TGT)
    inv = np.where(cnt > 0, 1.0 / np.maximum(cnt, 1), 0.0).astype(np.float32)
    bounds = np.searchsorted(sp, np.arange(0, N_TGT + 1, G))

    idx_rows, rel_cols, wgt_cols = [], [], []
    groups = []                # per group: list of (k, band_start, band_width)
    k = 0
    for j in range(N_TGT // G):
        lo, hi = int(bounds[j]), int(bounds[j + 1])
        rows = perm[lo:hi]
        loc = (sp[lo:hi] - j * G).astype(np.int64)   # local target ids in [0, G)
        L = hi - lo
        nsub = max(1, -(-L // P))
        pad = nsub * P - L
        rows_p = np.concatenate([rows, np.zeros(pad, np.int64)])
        loc_p = np.concatenate([loc, np.full(pad, -1, np.int64)])
        wgt_p = np.concatenate([inv[sp[lo:hi]], np.zeros(pad, np.float32)]).astype(np.float32)

        # narrow bands (psum window is memset to zero, so gaps are fine)
        bands = []
        for t in range(nsub):
            l = loc_p[t * P:(t + 1) * P]
            real = l >= 0
            if real.any():
                tmin, tmax = int(l[real].min()), int(l[real].max())
                if tmax - tmin < BW:
                    b, bw = min(tmin, G - BW), BW
                else:
                    b, bw = 0, G
            else:
                b, bw = 0, BW
            bands.append((b, bw))

        subs = []
        for t in range(nsub):
            b, bw = bands[t]
            l = loc_p[t * P:(t + 1) * P]
            rel = np.where(l >= 0, l - b, -1).astype(np.float32)
            idx_rows.append(rows_p[t * P:(t + 1) * P])
            rel_cols.append(rel)
            wgt_cols.append(wgt_p[t * P:(t + 1) * P])
            subs.append((k, b, bw))
            k += 1
        groups.append(subs)

    n_sub = k
    idx = np.stack(idx_rows)                      # [n_sub, 128] source rows (0 for pads)
    rw = np.zeros((P, 2 * n_sub), np.float32)
    rw[:, 0::2] = np.stack(rel_cols, axis=1)
    rw[:, 1::2] = np.stack(wgt_cols, axis=1)
    return idx, rw, groups, n_sub


def _build(n_sub, groups, x_bufs=4, repeats=1):
    nc = bacc.Bacc("TRN2", target_bir_lowering=False, debug=False, enable_asserts=False)
    FREE = n_sub * P
    xp = nc.dram_tensor("xp", [S_PER_CORE, P, FREE], F32, kind="ExternalInput").ap()
    rw = nc.dram_tensor("rw", [P, 2 * n_sub], F32, kind="ExternalInput").ap()
    wt = nc.dram_tensor("wt", [P, P], F32, kind="ExternalInput").ap()
    bi = nc.dram_tensor("bias", [P, 1], F32, kind="ExternalInput").ap()
    io = nc.dram_tensor("iota", [P, G], F32, kind="ExternalInput").ap()
    out = nc.dram_tensor("out", [S_PER_CORE, P, N_TGT], F32, kind="ExternalOutput").ap()

    eq, mu = mybir.AluOpType.is_equal, mybir.AluOpType.mult

    with ExitStack() as ctx:
        tc = ctx.enter_context(tile.TileContext(nc))
        cpool = ctx.enter_context(tc.tile_pool(name="const", bufs=1))
        xpool = ctx.enter_context(tc.tile_pool(name="x", bufs=x_bufs))
        apool = ctx.enter_context(tc.tile_pool(name="a", bufs=8))
        mpool = ctx.enter_context(tc.tile_pool(name="mean", bufs=4))
        opool = ctx.enter_context(tc.tile_pool(name="osb", bufs=2))
        pp = ctx.enter_context(tc.tile_pool(name="pp", bufs=4, space="PSUM"))
        po = ctx.enter_context(tc.tile_pool(name="po", bufs=3, space="PSUM"))

        rw_t = cpool.tile([P, 2 * n_sub], F32, tag="rw")
        nc.sync.dma_start(out=rw_t[:], in_=rw[:])
        wt_t = cpool.tile([P, P], F32, tag="wt")
        nc.sync.dma_start(out=wt_t[:], in_=wt[:])
        bi_t = cpool.tile([P, 1], F32, tag="bi")
        nc.sync.dma_start(out=bi_t[:], in_=bi[:])
        io_t = cpool.tile([P, G], F32, tag="io")
        nc.sync.dma_start(out=io_t[:], in_=io[:])

        for s in [s for _ in range(repeats) for s in range(S_PER_CORE)]:
            chunk_tiles = {}

            def get_chunk(c, s=s, chunk_tiles=chunk_tiles):
                t = chunk_tiles.get(c)
                if t is None:
                    f0 = c * CHUNK_SUBS * P
                    f1 = min(FREE, f0 + CHUNK_SUBS * P)
                    t = xpool.tile([P, f1 - f0], F32, tag="xc")
                    nc.sync.dma_start(out=t[:], in_=xp[s, :, f0:f1])
                    chunk_tiles[c] = t
                return t

            out_t = None
            for j, subs in enumerate(groups):
                if j % OUT_GROUPS == 0:
                    out_t = opool.tile([P, OUT_GROUPS * G], F32, tag="ot")
                ps = pp.tile([P, G], F32, tag="pp")
                nc.vector.memset(ps[:], 0.0)
                nlast = len(subs) - 1
                for t_i, (k, b, bw) in enumerate(subs):
                    c, off = divmod(k, CHUNK_SUBS)
                    xt = get_chunk(c)
                    a_t = apool.tile([P, bw], F32, tag="at")
                    nc.vector.tensor_scalar(
                        out=a_t[:], in0=io_t[:, :bw],
                        scalar1=rw_t[:, 2 * k:2 * k + 1],
                        scalar2=rw_t[:, 2 * k + 1:2 * k + 2],
                        op0=eq, op1=mu)
                    # start=False always: the window was zeroed by the memset, so
                    # accumulate-vs-overwrite on stale has_written bits is immaterial
                    nc.tensor.matmul(
                        out=ps[:, b:b + bw],
                        lhsT=xt[:, off * P:(off + 1) * P],
                        rhs=a_t[:],
                        start=False, stop=(t_i == nlast),
                        skip_group_check=True)
                mean_t = mpool.tile([P, G], F32, tag="mt")
                # DVE copy: [128,256] f32 is ~194ns on DVE vs ~2-9x on ACT
                nc.vector.tensor_copy(out=mean_t[:], in_=ps[:])
                pso = po.tile([P, G], F32, tag="po")
                nc.tensor.matmul(out=pso[:], lhsT=wt_t[:], rhs=mean_t[:],
                                 start=True, stop=True)
                nc.scalar.add(out_t[:, (j % OUT_GROUPS) * G:(j % OUT_GROUPS + 1) * G],
                              pso[:], bi_t[:, 0:1])
                if j % OUT_GROUPS == OUT_GROUPS - 1:
                    j0 = j - (OUT_GROUPS - 1)
                    nc.sync.dma_start(out=out[s, :, j0 * G:(j + 1) * G], in_=out_t[:])
    nc.compile()
    return nc


_CACHE = {}


def _prepare(parent_map):
    key = np.asarray(parent_map).astype(np.int64).tobytes()
    entry = _CACHE.get(key)
    if entry is None:
        idx, rw, groups, n_sub = _plan(parent_map)
        nc = _build(n_sub, groups)
        entry = (nc, idx, rw, n_sub)
        _CACHE[key] = entry
        _CACHE[(key, "plan")] = (idx, rw, groups, n_sub)
    return entry


def build_repeated(parent_map, repeats):
    """Benchmark variant: same program with the whole body repeated."""
    _prepare(parent_map)
    key = np.asarray(parent_map).astype(np.int64).tobytes()
    idx, rw, groups, n_sub = _CACHE[(key, "plan")]
    return _build(n_sub, groups, repeats=repeats)


def make_in_maps(x, parent_map, W, b):
    """Pack full inputs into the 8 per-core input maps."""
    nc, idx, rw, n_sub = _prepare(parent_map)
    x2 = np.ascontiguousarray(np.asarray(x, np.float32).reshape(S, N_SRC, D))
    FREE = n_sub * P
    flat_idx = idx.ravel()
    xp_all = np.empty((S, P, FREE), np.float32)
    for s in range(S):
        g = x2[s][flat_idx].reshape(n_sub, P, D)
        xp_all[s] = g.transpose(1, 0, 2).reshape(P, FREE)
    wt = np.ascontiguousarray(np.asarray(W, np.float32).T)
    bias = np.ascontiguousarray(np.asarray(b, np.float32).reshape(P, 1))
    iota = np.ascontiguousarray(
        np.broadcast_to(np.arange(G, dtype=np.float32), (P, G)))
    in_maps = []
    for c in range(N_CORES):
        in_maps.append({
            "xp": np.ascontiguousarray(xp_all[c * S_PER_CORE:(c + 1) * S_PER_CORE]),
            "rw": rw, "wt": wt, "bias": bias, "iota": iota,
        })
    return nc, in_maps


def assemble_output(results):
    """results: per-core list of {"out": [S_PER_CORE, P, N_TGT]} -> full output."""
    outs = np.stack([np.asarray(results[c]["out"]) for c in range(N_CORES)])
    out_full = outs.reshape(S, P, N_TGT).transpose(0, 2, 1)
    return np.ascontiguousarray(out_full).reshape(B, C, T, N_TGT, D).astype(np.float32)


def kernel(x, parent_map, W, b):
    nc, in_maps = make_in_maps(x, parent_map, W, b)
    res = run_bass_kernel_spmd(nc, in_maps, list(range(N_CORES)))
    return assemble_output(res.results)



# revision 2
# speedup vs baseline: 1.8508x; 1.8508x over previous
"""HEALPix downsample (scatter-mean over parent_map + 1x1 conv) on 8 Trainium2 cores.

Strategy (matches the data-parallel sharding hint):
  - Data-parallel over the 16 flattened (B*C*T) slices: 2 slices per core.
  - Host prep: sort source pixels by parent and pack x in that order as fp16
    (no padding -- a 128-row sub-tile that straddles a 512-target PSUM window
    boundary simply issues one matmul into each window). The scatter routing
    matrices A[row, tgt] = 1/cnt(parent) are precomputed on the host in fp16
    and loaded once (parent_map is replicated), so the device does no
    per-sub-tile index math at all.
  - Device: for each 512-target PSUM window, the first incident sub-tile does
    a full-width matmul with start=True (initializes the whole window; no
    memset needed), the rest accumulate into narrow bands. The 1x1 conv is a
    second matmul with W^T stationary; bias is fused into the PSUM->SBUF copy
    on the ScalarEngine. All matmul operands are fp16 (1 PE cycle/row vs 4
    for fp32), accumulation stays fp32 in PSUM.
  - x is streamed in big chunked DMAs; output is written as fp16 [e, m] per
    slice and transposed/upcast back on the host.
"""

import numpy as np
from contextlib import ExitStack

import concourse.bacc as bacc
import concourse.tile as tile
from concourse import mybir
from concourse.bass_utils import run_bass_kernel_spmd

# ---- problem constants (hardcoded; kernel.py must be self-contained) ----
B, C, T = 2, 2, 4
S = B * C * T                 # 16 flattened batch slices
N_SRC = 49152
N_TGT = 12288
D = 128
N_CORES = 8
S_PER_CORE = S // N_CORES     # 2

P = 128                       # partitions
G = 512                       # targets per psum window (one full PSUM bank)
N_SUB = N_SRC // P            # 384 sub-tiles per slice, no padding
N_WIN = N_TGT // G            # 24 windows per slice
CHUNK_SUBS = 96               # sub-tiles per x DMA chunk ([128, 12288] fp16 = 3 MB)
OUT_WINS = 4                  # windows staged per output DMA ([128, 2048] fp16 = 512 KB)

F32 = mybir.dt.float32
F16 = mybir.dt.float16


def _plan(parent_map):
    """Host-side metadata: sorted row order, per-window matmul schedule, and
    the concatenated fp16 A blocks."""
    pm = np.asarray(parent_map).astype(np.int64).ravel()
    assert pm.shape == (N_SRC,)
    perm = np.argsort(pm, kind="stable")
    sp = pm[perm]
    cnt = np.bincount(pm, minlength=N_TGT)
    inv = np.where(cnt > 0, 1.0 / np.maximum(cnt, 1), 0.0).astype(np.float32)
    inv_rows = inv[sp]                       # weight for each sorted source row

    # incidences[j] = list of (sub_tile, a_off, band_lo, band_w) for window j
    incidences = [[] for _ in range(N_WIN)]
    a_cols = []                              # growing list of [128, bw] blocks
    a_off = 0
    for t in range(N_SUB):
        loc_all = sp[t * P:(t + 1) * P]
        w = inv_rows[t * P:(t + 1) * P]
        jlo, jhi = int(loc_all[0] // G), int(loc_all[-1] // G)
        for j in range(jlo, jhi + 1):
            l = loc_all - j * G
            mask = (l >= 0) & (l < G)
            first = len(incidences[j]) == 0
            if first:
                b, bw = 0, G
            else:
                lm = l[mask]
                b = int(lm.min())
                bw = int(lm.max()) - b + 1
                bw = min(G - b, -(-bw // 8) * 8)   # round band to x8
            blk = np.zeros((P, bw), np.float16)
            rows = np.nonzero(mask)[0]
            blk[rows, l[rows] - b] = w[rows].astype(np.float16)
            a_cols.append(blk)
            incidences[j].append((t, a_off, b, bw))
            a_off += bw
    a_all = np.concatenate(a_cols, axis=1)   # [128, a_off] fp16
    return perm, incidences, np.ascontiguousarray(a_all)


def _build(incidences, a_total, repeats=1):
    nc = bacc.Bacc("TRN2", target_bir_lowering=False, debug=False, enable_asserts=False)
    FREE = N_SUB * P
    xp = nc.dram_tensor("xp", [S_PER_CORE, P, FREE], F16, kind="ExternalInput").ap()
    am = nc.dram_tensor("am", [P, a_total], F16, kind="ExternalInput").ap()
    wt = nc.dram_tensor("wt", [P, P], F16, kind="ExternalInput").ap()
    bi = nc.dram_tensor("bias", [P, 1], F32, kind="ExternalInput").ap()
    out = nc.dram_tensor("out", [S_PER_CORE, P, N_TGT], F16, kind="ExternalOutput").ap()

    with ExitStack() as ctx:
        tc = ctx.enter_context(tile.TileContext(nc))
        cpool = ctx.enter_context(tc.tile_pool(name="const", bufs=1))
        xpool = ctx.enter_context(tc.tile_pool(name="x", bufs=3))
        mpool = ctx.enter_context(tc.tile_pool(name="mean", bufs=4))
        opool = ctx.enter_context(tc.tile_pool(name="osb", bufs=2))
        pp = ctx.enter_context(tc.tile_pool(name="pp", bufs=3, space="PSUM"))
        po = ctx.enter_context(tc.tile_pool(name="po", bufs=2, space="PSUM"))

        am_t = cpool.tile([P, a_total], F16, tag="am")
        nc.sync.dma_start(out=am_t[:], in_=am[:])
        wt_t = cpool.tile([P, P], F16, tag="wt")
        nc.sync.dma_start(out=wt_t[:], in_=wt[:])
        bi_t = cpool.tile([P, 1], F32, tag="bi")
        nc.sync.dma_start(out=bi_t[:], in_=bi[:])

        for s in [s for _ in range(repeats) for s in range(S_PER_CORE)]:
            chunk_tiles = {}

            def get_chunk(c, s=s, chunk_tiles=chunk_tiles):
                t = chunk_tiles.get(c)
                if t is None:
                    f0 = c * CHUNK_SUBS * P
                    f1 = min(FREE, f0 + CHUNK_SUBS * P)
                    t = xpool.tile([P, f1 - f0], F16, tag="xc")
                    nc.sync.dma_start(out=t[:], in_=xp[s, :, f0:f1])
                    chunk_tiles[c] = t
                return t

            out_t = None
            for j in range(N_WIN):
                if j % OUT_WINS == 0:
                    out_t = opool.tile([P, OUT_WINS * G], F16, tag="ot")
                ps = pp.tile([P, G], F32, tag="pp")
                subs = incidences[j]
                if not subs:
                    nc.vector.memset(ps[:], 0.0)
                nlast = len(subs) - 1
                for i, (t_sub, a_off, b, bw) in enumerate(subs):
                    c, off = divmod(t_sub, CHUNK_SUBS)
                    xt = get_chunk(c)
                    nc.tensor.matmul(
                        out=ps[:, b:b + bw],
                        lhsT=xt[:, off * P:(off + 1) * P],
                        rhs=am_t[:, a_off:a_off + bw],
                        start=(i == 0), stop=(i == nlast),
                        skip_group_check=True)
                mean_t = mpool.tile([P, G], F16, tag="mt")
                nc.vector.tensor_copy(out=mean_t[:], in_=ps[:])
                pso = po.tile([P, G], F32, tag="po")
                nc.tensor.matmul(out=pso[:], lhsT=wt_t[:], rhs=mean_t[:],
                                 start=True, stop=True)
                nc.scalar.add(out_t[:, (j % OUT_WINS) * G:(j % OUT_WINS + 1) * G],
                              pso[:], bi_t[:, 0:1])
                if j % OUT_WINS == OUT_WINS - 1:
                    j0 = j - (OUT_WINS - 1)
                    nc.sync.dma_start(out=out[s, :, j0 * G:(j + 1) * G], in_=out_t[:])
    nc.compile()
    return nc


_CACHE = {}


def _prepare(parent_map):
    key = np.asarray(parent_map).astype(np.int64).tobytes()
    entry = _CACHE.get(key)
    if entry is None:
        perm, incidences, a_all = _plan(parent_map)
        nc = _build(incidences, a_all.shape[1])
        entry = (nc, perm, a_all)
        _CACHE[key] = entry
        _CACHE[(key, "plan")] = (perm, incidences, a_all)
    return entry


def build_repeated(parent_map, repeats):
    """Benchmark variant: same program with the whole body repeated."""
    _prepare(parent_map)
    key = np.asarray(parent_map).astype(np.int64).tobytes()
    perm, incidences, a_all = _CACHE[(key, "plan")]
    return _build(incidences, a_all.shape[1], repeats=repeats)


def make_in_maps(x, parent_map, W, b):
    """Pack full inputs into the 8 per-core input maps."""
    nc, perm, a_all = _prepare(parent_map)
    x2 = np.asarray(x, np.float32).reshape(S, N_SRC, D)
    FREE = N_SUB * P
    xp_all = np.empty((S, P, FREE), np.float16)
    for s in range(S):
        g = x2[s][perm].astype(np.float16).reshape(N_SUB, P, D)
        xp_all[s] = g.transpose(1, 0, 2).reshape(P, FREE)
    wt = np.ascontiguousarray(np.asarray(W, np.float32).T.astype(np.float16))
    bias = np.ascontiguousarray(np.asarray(b, np.float32).reshape(P, 1))
    in_maps = []
    for c in range(N_CORES):
        in_maps.append({
            "xp": np.ascontiguousarray(xp_all[c * S_PER_CORE:(c + 1) * S_PER_CORE]),
            "am": a_all, "wt": wt, "bias": bias,
        })
    return nc, in_maps


def assemble_output(results):
    """results: per-core list of {"out": [S_PER_CORE, P, N_TGT]} -> full output."""
    outs = np.stack([np.asarray(results[c]["out"]) for c in range(N_CORES)])
    out_full = outs.reshape(S, P, N_TGT).transpose(0, 2, 1).astype(np.float32)
    return np.ascontiguousarray(out_full).reshape(B, C, T, N_TGT, D)


def kernel(x, parent_map, W, b):
    nc, in_maps = make_in_maps(x, parent_map, W, b)
    res = run_bass_kernel_spmd(nc, in_maps, list(range(N_CORES)))
    return assemble_output(res.results)


# revision 9
# speedup vs baseline: 2.7893x; 1.5071x over previous
"""HEALPix downsample (scatter-mean over parent_map + 1x1 conv) on 8 Trainium2 cores.

Strategy (matches the data-parallel sharding hint):
  - Data-parallel over the 16 flattened (B*C*T) slices: 2 slices per core.
  - Host prep: sort source pixels by parent and pack x in that order as fp16
    (no padding -- a 128-row sub-tile that straddles a 512-target PSUM window
    boundary simply issues one matmul into each window). The scatter routing
    matrices A[row, tgt] = 1/cnt(parent) are precomputed on the host in fp16
    and loaded once (parent_map is replicated), so the device does no
    per-sub-tile index math at all.
  - Device: each 512-target PSUM window is initialized by a full-width
    matmul against a shared zero A-block with start=True (no memset needed),
    then incident sub-tiles accumulate into narrow bands. The 1x1 conv is a
    second matmul with W^T stationary; bias is fused into the PSUM->SBUF copy
    on the ScalarEngine. All matmul operands are fp16 (1 PE cycle/row vs 4
    for fp32), accumulation stays fp32 in PSUM.
  - x is streamed in big chunked DMAs; output is written as fp16 [e, m] per
    slice and transposed/upcast back on the host.
"""

import numpy as np
from contextlib import ExitStack

import concourse.bacc as bacc
import concourse.tile as tile
from concourse import mybir
from concourse.bass_utils import run_bass_kernel_spmd

# ---- problem constants (hardcoded; kernel.py must be self-contained) ----
B, C, T = 2, 2, 4
S = B * C * T                 # 16 flattened batch slices
N_SRC = 49152
N_TGT = 12288
D = 128
N_CORES = 8
S_PER_CORE = S // N_CORES     # 2

P = 128                       # partitions
G = 512                       # targets per psum window (one full PSUM bank)
N_SUB = N_SRC // P            # 384 sub-tiles per slice, no padding
N_WIN = N_TGT // G            # 24 windows per slice
CHUNK_SUBS = 128              # sub-tiles per x DMA chunk ([128, 16384] fp16 = 4 MB)
OUT_WINS = 12                 # windows staged per output DMA ([128, 6144] fp16 = 1.5 MB)

F32 = mybir.dt.float32
F16 = mybir.dt.float16

# Splitting DMAs across both HWDGE rings (SP + Activation) measured SLOWER
# than a single SP ring (95.0us vs 87.0us per body for the same bytes), so
# everything goes through nc.sync.
DMA_SPLIT = False


def _plan(parent_map):
    """Host-side metadata: sorted row order, per-window matmul schedule, and
    the concatenated fp16 A blocks."""
    pm = np.asarray(parent_map).astype(np.int64).ravel()
    assert pm.shape == (N_SRC,)
    perm = np.argsort(pm, kind="stable")
    sp = pm[perm]
    cnt = np.bincount(pm, minlength=N_TGT)
    inv = np.where(cnt > 0, 1.0 / np.maximum(cnt, 1), 0.0).astype(np.float32)
    inv_rows = inv[sp]                       # weight for each sorted source row

    # incidences[j] = list of (sub_tile, a_off, band_lo, band_w) for window j.
    # Block 0 of the A matrix is a shared [128, G] zero block: the first
    # matmul of every window uses it full-width with start=True to initialize
    # the whole PSUM window (contributes x*0), then banded blocks accumulate.
    incidences = [[] for _ in range(N_WIN)]
    a_cols = [np.zeros((P, G), np.float16)]  # shared zero block at a_off=0
    a_off = G
    for t in range(N_SUB):
        loc_all = sp[t * P:(t + 1) * P]
        w = inv_rows[t * P:(t + 1) * P]
        jlo, jhi = int(loc_all[0] // G), int(loc_all[-1] // G)
        for j in range(jlo, jhi + 1):
            l = loc_all - j * G
            mask = (l >= 0) & (l < G)
            if len(incidences[j]) == 0:
                incidences[j].append((t, 0, 0, G))
            lm = l[mask]
            b = int(lm.min())
            bw = int(lm.max()) - b + 1
            bw = min(G - b, -(-bw // 8) * 8)   # round band to x8
            blk = np.zeros((P, bw), np.float16)
            rows = np.nonzero(mask)[0]
            blk[rows, l[rows] - b] = w[rows].astype(np.float16)
            a_cols.append(blk)
            incidences[j].append((t, a_off, b, bw))
            a_off += bw
    a_all = np.concatenate(a_cols, axis=1)   # [128, a_off] fp16
    return perm, incidences, np.ascontiguousarray(a_all)


def _build(incidences, a_total, repeats=1):
    nc = bacc.Bacc("TRN2", target_bir_lowering=False, debug=False, enable_asserts=False)
    FREE = N_SUB * P
    xp = nc.dram_tensor("xp", [S_PER_CORE, P, FREE], F16, kind="ExternalInput").ap()
    am = nc.dram_tensor("am", [P, a_total], F16, kind="ExternalInput").ap()
    wt = nc.dram_tensor("wt", [P, P], F16, kind="ExternalInput").ap()
    bi = nc.dram_tensor("bias", [P, 1], F32, kind="ExternalInput").ap()
    out = nc.dram_tensor("out", [S_PER_CORE, P, N_TGT], F16, kind="ExternalOutput").ap()

    with ExitStack() as ctx:
        tc = ctx.enter_context(tile.TileContext(nc))
        cpool = ctx.enter_context(tc.tile_pool(name="const", bufs=1))
        xpool = ctx.enter_context(tc.tile_pool(name="x", bufs=3))
        mpool = ctx.enter_context(tc.tile_pool(name="mean", bufs=4))
        opool = ctx.enter_context(tc.tile_pool(name="osb", bufs=2))
        pp = ctx.enter_context(tc.tile_pool(name="pp", bufs=3, space="PSUM"))
        po = ctx.enter_context(tc.tile_pool(name="po", bufs=2, space="PSUM"))

        am_t = cpool.tile([P, a_total], F16, tag="am")
        nc.sync.dma_start(out=am_t[:], in_=am[:])
        wt_t = cpool.tile([P, P], F16, tag="wt")
        nc.sync.dma_start(out=wt_t[:], in_=wt[:])
        bi_t = cpool.tile([P, 1], F32, tag="bi")
        nc.sync.dma_start(out=bi_t[:], in_=bi[:])

        n_dma = [0]

        def in_engine():
            n_dma[0] += 1
            return nc.scalar if (DMA_SPLIT and n_dma[0] % 2) else nc.sync

        for s in [s for _ in range(repeats) for s in range(S_PER_CORE)]:
            chunk_tiles = {}

            def get_chunk(c, s=s, chunk_tiles=chunk_tiles):
                t = chunk_tiles.get(c)
                if t is None:
                    f0 = c * CHUNK_SUBS * P
                    f1 = min(FREE, f0 + CHUNK_SUBS * P)
                    t = xpool.tile([P, f1 - f0], F16, tag="xc")
                    in_engine().dma_start(out=t[:], in_=xp[s, :, f0:f1])
                    chunk_tiles[c] = t
                return t

            out_t = None
            for j in range(N_WIN):
                if j % OUT_WINS == 0:
                    out_t = opool.tile([P, OUT_WINS * G], F16, tag="ot")
                ps = pp.tile([P, G], F32, tag="pp")
                subs = incidences[j]
                if not subs:
                    nc.vector.memset(ps[:], 0.0)
                nlast = len(subs) - 1
                for i, (t_sub, a_off, b, bw) in enumerate(subs):
                    c, off = divmod(t_sub, CHUNK_SUBS)
                    xt = get_chunk(c)
                    nc.tensor.matmul(
                        out=ps[:, b:b + bw],
                        lhsT=xt[:, off * P:(off + 1) * P],
                        rhs=am_t[:, a_off:a_off + bw],
                        start=(i == 0), stop=(i == nlast),
                        skip_group_check=True)
                mean_t = mpool.tile([P, G], F16, tag="mt")
                nc.vector.tensor_copy(out=mean_t[:], in_=ps[:])
                pso = po.tile([P, G], F32, tag="po")
                nc.tensor.matmul(out=pso[:], lhsT=wt_t[:], rhs=mean_t[:],
                                 start=True, stop=True)
                nc.scalar.add(out_t[:, (j % OUT_WINS) * G:(j % OUT_WINS + 1) * G],
                              pso[:], bi_t[:, 0:1])
                if j % OUT_WINS == OUT_WINS - 1:
                    j0 = j - (OUT_WINS - 1)
                    in_engine().dma_start(out=out[s, :, j0 * G:(j + 1) * G],
                                          in_=out_t[:])
    nc.compile()
    return nc


_CACHE = {}


def _prepare(parent_map):
    key = np.asarray(parent_map).astype(np.int64).tobytes()
    entry = _CACHE.get(key)
    if entry is None:
        perm, incidences, a_all = _plan(parent_map)
        nc = _build(incidences, a_all.shape[1])
        entry = (nc, perm, a_all)
        _CACHE[key] = entry
        _CACHE[(key, "plan")] = (perm, incidences, a_all)
    return entry


def build_repeated(parent_map, repeats):
    """Benchmark variant: same program with the whole body repeated."""
    _prepare(parent_map)
    key = np.asarray(parent_map).astype(np.int64).tobytes()
    perm, incidences, a_all = _CACHE[(key, "plan")]
    return _build(incidences, a_all.shape[1], repeats=repeats)


def make_in_maps(x, parent_map, W, b):
    """Pack full inputs into the 8 per-core input maps."""
    nc, perm, a_all = _prepare(parent_map)
    x2 = np.asarray(x, np.float32).reshape(S, N_SRC, D)
    FREE = N_SUB * P
    xp_all = np.empty((S, P, FREE), np.float16)
    for s in range(S):
        g = x2[s][perm].astype(np.float16).reshape(N_SUB, P, D)
        xp_all[s] = g.transpose(1, 0, 2).reshape(P, FREE)
    wt = np.ascontiguousarray(np.asarray(W, np.float32).T.astype(np.float16))
    bias = np.ascontiguousarray(np.asarray(b, np.float32).reshape(P, 1))
    in_maps = []
    for c in range(N_CORES):
        in_maps.append({
            "xp": np.ascontiguousarray(xp_all[c * S_PER_CORE:(c + 1) * S_PER_CORE]),
            "am": a_all, "wt": wt, "bias": bias,
        })
    return nc, in_maps


def assemble_output(results):
    """results: per-core list of {"out": [S_PER_CORE, P, N_TGT]} -> full output."""
    outs = np.stack([np.asarray(results[c]["out"]) for c in range(N_CORES)])
    out_full = outs.reshape(S, P, N_TGT).transpose(0, 2, 1).astype(np.float32)
    return np.ascontiguousarray(out_full).reshape(B, C, T, N_TGT, D)


def kernel(x, parent_map, W, b):
    nc, in_maps = make_in_maps(x, parent_map, W, b)
    res = run_bass_kernel_spmd(nc, in_maps, list(range(N_CORES)))
    return assemble_output(res.results)
